# revision 27
# baseline (speedup 1.0000x reference)
"""Trainium2 Bass kernel for nn_AttentionModel2 (Kool-style attention encoder).

Model (per reference):
  h = x @ Wemb + bemb                      # [B=256, N=200, D=128]
  3 layers of:
    h = BN1(h + MHA(h))                    # BatchNorm1d training mode (global stats)
    h = BN2(h + FFN(BN1-output))           # FFN = relu(h W1 + b1) W2 + b2
  return (h, h.mean(axis=1))

Sharding: data-parallel over batch, 32 instances per core x 8 cores.
BatchNorm statistics are global over all 256*200 tokens -> tiny AllGather of
per-core (sum, sumsq) partials, 6x (2 BN per layer).

Layout: activations live transposed [D=128 partitions, 6400 tokens] on-chip.
b2 is dropped entirely: a per-channel constant shift cancels exactly in BN2.
"""

import os
import sys
import numpy as np

B, N, NODE_DIM = 256, 200, 2
H, D, L, FF = 8, 128, 3, 512
KD = D // H  # 16
EPS = 1e-5
NCORES = 8
BL = B // NCORES          # 32 instances per core
T = BL * N                # 6400 tokens per core
NQ = 8                    # quads (4 instances each)
TQ = 4 * N                # 800 tokens per quad

_BUILD_CACHE = {}


def _build_bass():
    PHASE = int(os.environ.get("KERNEL_PHASE", "8"))
    sys.path.insert(0, "/opt/trn_rl_repo")
    import concourse.bass as bass
    import concourse.mybir as mybir
    import concourse.tile as tile
    from concourse import bacc

    F32 = mybir.dt.float32
    BF16 = mybir.dt.float16  # fp16 for weights/activations (better mantissa)
    EX16 = mybir.dt.bfloat16  # bf16 where range matters (expS can reach e^85)
    F32R = mybir.dt.float32r
    AF = mybir.ActivationFunctionType
    OP = mybir.AluOpType

    nc = bacc.Bacc("TRN2", target_bir_lowering=False)

    # ---------------- DRAM I/O ----------------
    d_x = nc.dram_tensor("x", [NODE_DIM, T], F32, kind="ExternalInput")
    d_wemb = nc.dram_tensor("wemb", [NODE_DIM, D], F32, kind="ExternalInput")
    d_bemb = nc.dram_tensor("bemb", [D, 1], F32, kind="ExternalInput")
    F16_ = mybir.dt.float16
    d_wqp = nc.dram_tensor("wqp", [L, 2, D, 128], F16_, kind="ExternalInput")
    d_wkp = nc.dram_tensor("wkp", [L, 2, D, 128], F16_, kind="ExternalInput")
    d_wvd = nc.dram_tensor("wvd", [L, D, 128], F16_, kind="ExternalInput")
    d_wop = nc.dram_tensor("wop", [L, 2, 128, D], F16_, kind="ExternalInput")
    d_w1 = nc.dram_tensor("w1", [L, D, FF], F16_, kind="ExternalInput")
    d_w2 = nc.dram_tensor("w2", [L, 4, 128, D], F16_, kind="ExternalInput")
    d_b1t = nc.dram_tensor("b1t", [L, 128, 4], F32, kind="ExternalInput")
    d_bnp = nc.dram_tensor("bnp", [L, D, 4], F32, kind="ExternalInput")

    d_oh = nc.dram_tensor("oh", [D, T], F32, kind="ExternalOutput")
    d_om = nc.dram_tensor("om", [D, BL], F32, kind="ExternalOutput")

    # collective buffers (one pair per BN instance)
    cc_in = [nc.dram_tensor(f"cc_in{k}", [1, 2 * D], F32) for k in range(2 * L)]
    cc_out = [
        nc.dram_tensor(f"cc_out{k}", [NCORES, 2 * D], F32, addr_space="Shared")
        for k in range(2 * L)
    ]
    # scratch for softmax denominators (per instance: 4 rows x 400)
    d_dscr = nc.dram_tensor("dscr", [BL, 4, 400], F32)

    # ---------------- persistent SBUF ----------------
    def sb(name, shape, dt):
        return nc.alloc_sbuf_tensor(name, shape, dt).ap()

    XTn = sb("XTn", [D, T], F32)           # spine (normalized h.T)
    XS = sb("XS", [D, T], BF16)            # bf16 shadow for matmul rhs
    HP1 = sb("HP1", [D, T], F32)           # pre-BN spine
    VPall = sb("VPall", [128, 2 * BL * 136], EX16)  # V' per (b,c): 8 heads x 17
    xS2 = sb("xS2", [NODE_DIM, T], F32)

    w_emb = sb("w_emb", [NODE_DIM, D], F32)
    w_qp = [[sb(f"w_qp{l}_{g}", [D, 128], BF16) for g in range(2)] for l in range(L)]
    w_kp = [[sb(f"w_kp{l}_{g}", [D, 128], BF16) for g in range(2)] for l in range(L)]
    w_vd = [sb(f"w_vd{l}", [D, 128], BF16) for l in range(L)]
    w_op = [[sb(f"w_op{l}_{g}", [128, D], BF16) for g in range(2)] for l in range(L)]
    w_1 = [sb(f"w_1_{l}", [D, FF], BF16) for l in range(L)]
    w_2 = [[sb(f"w_2_{l}_{c}", [128, D], BF16) for c in range(4)] for l in range(L)]

    c_bemb = sb("c_bemb", [D, 1], F32)
    c_b1 = [sb(f"c_b1_{l}", [128, 4], F32) for l in range(L)]
    c_bnp = [sb(f"c_bnp_{l}", [D, 4], F32) for l in range(L)]

    # small stat tensors
    s_stats = sb("s_stats", [D, 16, 6], F32)
    s_mv = sb("s_mv", [D, 2], F32)
    s_sums = sb("s_sums", [D, 2], F32)
    s_tmp = sb("s_tmp", [D, 2], F32)
    s_gat = sb("s_gat", [D, NCORES, 2], F32)
    s_gl = sb("s_gl", [D, 2], F32)         # global (sum, sumsq) -> (mean, var)
    s_sc = sb("s_sc", [D, 1], F32)         # BN scale
    s_sh = sb("s_sh", [D, 1], F32)         # BN shift
    s_mean = sb("s_mean", [D, BL], F32)
    s_eps = sb("s_eps", [D, 1], F32)

    CHUNKS = [(i * 512, 512) for i in range(12)] + [(6144, 256)]

    from contextlib import ExitStack
    _es = ExitStack()
    with tile.TileContext(nc) as tc:
        psA = _es.enter_context(tc.tile_pool(name="psA", bufs=1, space="PSUM"))
        psB = _es.enter_context(tc.tile_pool(name="psB", bufs=2, space="PSUM"))
        psH = _es.enter_context(tc.tile_pool(name="psH", bufs=2, space="PSUM"))
        sbp = _es.enter_context(tc.tile_pool(name="sbp", bufs=2))
        dpp = _es.enter_context(tc.tile_pool(name="dpp", bufs=4))
        exp_pool = _es.enter_context(tc.tile_pool(name="exp_pool", bufs=3))
        rlp = _es.enter_context(tc.tile_pool(name="rlp", bufs=1))

        # ---------- load weights ----------
        def load_round(dram_ap, sbuf_ap, cols):
            nc.sync.dma_start(out=sbuf_ap, in_=dram_ap)

        load_round(d_wemb[:], w_emb, D)
        for l in range(L):
            for g in range(2):
                load_round(d_wqp[l, g], w_qp[l][g], 128)
                load_round(d_wkp[l, g], w_kp[l][g], 128)
                load_round(d_wop[l, g], w_op[l][g], D)
            load_round(d_wvd[l], w_vd[l], 128)
            load_round(d_w1[l], w_1[l], FF)
            for c in range(4):
                load_round(d_w2[l, c], w_2[l][c], D)
            nc.sync.dma_start(out=c_b1[l], in_=d_b1t[l])
            nc.sync.dma_start(out=c_bnp[l], in_=d_bnp[l])
        nc.sync.dma_start(out=c_bemb[:], in_=d_bemb[:])

        nc.vector.memset(s_eps[:], EPS)
        # zero-init all PSUM pool slots (dead rows of M=17 col-tiled matmuls
        # are never written; stale Inf would poison Wo via 0*Inf)
        for _pool, _shape, _n in ((psA, [128, 1024], 1), (psB, [128, 1024], 2),
                                  (psH, [128, 400], 2)):
            for _i in range(_n):
                _pt = _pool.tile(_shape, F32, tag="sc" if _pool is psA else
                                 ("qkv" if _pool is psB else "h"))
                nc.vector.memset(_pt[:], 0.0)
        # V' ones columns (one-time; data evacs never touch col 16 of each 17)
        ones_ap = bass.AP(
            tensor=VPall.tensor,
            offset=VPall.offset + 16,
            ap=[[VPall.ap[0][0], 128], [136, 2 * BL], [17, 8]],
        )
        nc.vector.memset(ones_ap, 1.0)

        # ---------- embed ----------
        nc.sync.dma_start(out=xS2[:], in_=d_x[:])
        for (off, sz) in CHUNKS:
            pe = psB.tile([128, 1024], F32, tag="qkv")
            nc.tensor.matmul(pe[:, 0:sz], w_emb, xS2[:, off:off + sz],
                             start=True, stop=True)
            nc.vector.tensor_scalar(out=XTn[:, off:off + sz], in0=pe[:, 0:sz],
                                    scalar1=c_bemb[:, 0:1], scalar2=None,
                                    op0=OP.add)
            nc.vector.tensor_scalar(out=XS[:, off:off + sz], in0=pe[:, 0:sz],
                                    scalar1=c_bemb[:, 0:1], scalar2=None,
                                    op0=OP.add)

        # ---------- BN helper ----------
        def quad_stats(q):
            # incremental stats on HP1 quad (800 = 512 + 288)
            t0 = q * TQ
            nc.vector.bn_stats(s_stats[:, 2 * q, :], HP1[:, t0:t0 + 512])
            nc.vector.bn_stats(s_stats[:, 2 * q + 1, :], HP1[:, t0 + 512:t0 + 800])

        def bn_sync(l, which, src_ap, k):
            nc.vector.bn_aggr(s_mv[:], s_stats[:])
            # partial (sum, sumsq)
            nc.vector.tensor_scalar(out=s_sums[:, 0:1], in0=s_mv[:, 0:1],
                                    scalar1=float(T), scalar2=None, op0=OP.mult)
            nc.vector.tensor_tensor(out=s_tmp[:, 0:1], in0=s_mv[:, 0:1],
                                    in1=s_mv[:, 0:1], op=OP.mult)
            nc.vector.tensor_tensor(out=s_tmp[:, 1:2], in0=s_mv[:, 1:2],
                                    in1=s_tmp[:, 0:1], op=OP.add)
            nc.vector.tensor_scalar(out=s_sums[:, 1:2], in0=s_tmp[:, 1:2],
                                    scalar1=float(T), scalar2=None, op0=OP.mult)
            # DMA out [D,2] -> [1, 2D] (d-major)
            src = bass.AP(tensor=s_sums.tensor, offset=s_sums.offset,
                          ap=[[2, D], [1, 2]])
            nc.sync.dma_start(out=cc_in[k][:], in_=src)
            nc.gpsimd.collective_compute(
                "AllGather", OP.bypass,
                ins=[cc_in[k][:]], outs=[cc_out[k][:]],
                replica_groups=[list(range(NCORES))],
            )
            gsrc = bass.AP(tensor=cc_out[k][:].tensor, offset=cc_out[k][:].offset,
                           ap=[[2, D], [2 * D, NCORES], [1, 2]])
            nc.sync.dma_start(out=s_gat[:], in_=gsrc)
            nc.vector.tensor_reduce(out=s_gl[:],
                                    in_=s_gat[:].rearrange("p c s -> p s c"),
                                    axis=mybir.AxisListType.X, op=OP.add)
            inv = 1.0 / (B * N)
            nc.vector.tensor_scalar(out=s_gl[:, 0:1], in0=s_gl[:, 0:1],
                                    scalar1=inv, scalar2=None, op0=OP.mult)
            nc.vector.tensor_scalar(out=s_gl[:, 1:2], in0=s_gl[:, 1:2],
                                    scalar1=inv, scalar2=None, op0=OP.mult)
            # var = E[x^2] - mean^2
            nc.vector.tensor_tensor(out=s_tmp[:, 0:1], in0=s_gl[:, 0:1],
                                    in1=s_gl[:, 0:1], op=OP.mult)
            nc.vector.tensor_tensor(out=s_tmp[:, 1:2], in0=s_gl[:, 1:2],
                                    in1=s_tmp[:, 0:1], op=OP.subtract)
            # rstd = exp(-0.5*ln(var+eps)) (stays in natural_log_exp table set)
            nc.scalar.activation(out=s_tmp[:, 1:2], in_=s_tmp[:, 1:2],
                                 func=AF.Ln, bias=s_eps[:, 0:1], scale=1.0)
            nc.scalar.activation(out=s_tmp[:, 1:2], in_=s_tmp[:, 1:2],
                                 func=AF.Exp, bias=0.0, scale=-0.5)
            gcol = 0 if which == 1 else 2
            nc.vector.tensor_tensor(out=s_sc[:, 0:1], in0=s_tmp[:, 1:2],
                                    in1=c_bnp[l][:, gcol:gcol + 1], op=OP.mult)
            nc.vector.tensor_tensor(out=s_sh[:, 0:1], in0=s_gl[:, 0:1],
                                    in1=s_sc[:, 0:1], op=OP.mult)
            nc.vector.tensor_tensor(out=s_sh[:, 0:1],
                                    in0=c_bnp[l][:, gcol + 1:gcol + 2],
                                    in1=s_sh[:, 0:1], op=OP.subtract)
            # normalize per quad: XS (f16 shadow) first to unblock matmuls,
            # then XTn (fp32 spine)
            for qq in range(NQ):
                o0 = qq * TQ
                nc.vector.tensor_scalar(out=XS[:, o0:o0 + TQ],
                                        in0=src_ap[:, o0:o0 + TQ],
                                        scalar1=s_sc[:, 0:1], scalar2=s_sh[:, 0:1],
                                        op0=OP.mult, op1=OP.add)
                nc.vector.tensor_scalar(out=XTn[:, o0:o0 + TQ],
                                        in0=src_ap[:, o0:o0 + TQ],
                                        scalar1=s_sc[:, 0:1], scalar2=s_sh[:, 0:1],
                                        op0=OP.mult, op1=OP.add)

        # ---------- layers ----------
        for l in range(L if PHASE >= 8 else 1):
            # ===== attention =====
            for q in range(NQ if PHASE >= 1 else 0):
                t0 = q * TQ
                # Q, K projection (padded 4-head groups)
                QK = {}
                for (nm, wg) in (("q", w_qp[l]), ("k", w_kp[l])):
                    for g in range(2):
                        pp = psB.tile([128, 1024], F32, tag="qkv")
                        nc.tensor.matmul(pp[:, 0:512], wg[g], XS[:, t0:t0 + 512],
                                         start=True, stop=True)
                        nc.tensor.matmul(pp[:, 512:800], wg[g],
                                         XS[:, t0 + 512:t0 + 800],
                                         start=True, stop=True)
                        sbt = sbp.tile([128, 800], BF16, tag=f"{nm}{g}")
                        nc.vector.tensor_copy(sbt[:], pp[:, 0:800])
                        QK[(nm, g)] = sbt
                # V' production
                for bh in range(4):
                    b = 4 * q + bh
                    pv = psB.tile([128, 256], F32, tag="qkv")
                    for c in range(2):
                        kl = 128 if c == 0 else 72
                        nc.tensor.matmul(
                            pv[0:kl, 128 * c:128 * c + 128],
                            XS[:, t0 + 200 * bh + 128 * c:
                               t0 + 200 * bh + 128 * c + kl],
                            w_vd[l], start=True, stop=True)
                    for c in range(2):
                        kl = 128 if c == 0 else 72
                        slot = 2 * b + c
                        dst = bass.AP(
                            tensor=VPall.tensor,
                            offset=VPall.offset + slot * 136,
                            ap=[[VPall.ap[0][0], kl], [17, 8], [1, 16]])
                        src = pv[0:kl, 128 * c:128 * c + 128].rearrange(
                            "p (h k) -> p h k", h=8)
                        nc.vector.tensor_copy(dst, src)

                sq = sbp.tile([128, 1600], BF16, tag="sq")
                for bh in range(4 if PHASE >= 2 else 0):
                    b = 4 * q + bh
                    Hb = psH.tile([128, 400], F32, tag="h")
                    for g in range(2):
                        exs = []
                        for jp in range(2):
                            sc = psA.tile([128, 1024], F32, tag="sc")
                            for jj in range(2):
                                j = 2 * jp + jj
                                for c in range(2):
                                    kl = 128 if c == 0 else 72
                                    nc.tensor.matmul(
                                        sc[0:kl, 512 * jj + 200 * c:
                                           512 * jj + 200 * c + 200],
                                        QK[("k", g)][32 * j:32 * j + 16,
                                                     200 * bh + 128 * c:
                                                     200 * bh + 128 * c + kl],
                                        QK[("q", g)][32 * j:32 * j + 16,
                                                     200 * bh:200 * bh + 200],
                                        start=True, stop=True,
                                        tile_position=(32 * j, 0))
                            ex = exp_pool.tile([128, 2, 2, 200], EX16, tag="ex")
                            sc0 = sc[:]
                            esrc = bass.AP(
                                tensor=sc0.tensor, offset=sc0.offset,
                                ap=[list(sc0.ap[0]), [512, 2], [200, 2], [1, 200]])
                            nc.scalar.activation(out=ex[:], in_=esrc, func=AF.Exp)
                            exs.append(ex)
                        # attnV: c-major rounds so drains overlap across heads
                        for c in range(2 if PHASE >= 3 else 0):
                            kl = 128 if c == 0 else 72
                            slot = 2 * b + c
                            for jp in range(2):
                                for jj in range(2):
                                    j = 2 * jp + jj
                                    vap = bass.AP(
                                        tensor=VPall.tensor,
                                        offset=VPall.offset + slot * 136
                                        + 17 * (4 * g + j),
                                        ap=[[VPall.ap[0][0], kl], [1, 17]])
                                    nc.tensor.matmul(
                                        Hb[32 * j:32 * j + 17,
                                           200 * g:200 * g + 200],
                                        vap, exs[jp][0:kl, jj, c, :],
                                        start=(c == 0), stop=(c == 1),
                                        tile_position=(0, 32 * j))
                    if PHASE < 4:
                        continue
                    # free Hb fast: copy to SBUF, chain runs off the copy
                    hc = dpp.tile([128, 400], F32, tag="hc")
                    nc.vector.tensor_copy(hc[:], Hb[:])
                    dw = dpp.tile([128, 400], F32, tag="draw")
                    nc.vector.reciprocal_approx_fast(dw[:], hc[:])
                    dw0 = dw[:]
                    gsrc = bass.AP(tensor=dw0.tensor,
                                   offset=dw0.offset + 16 * dw0.ap[0][0],
                                   ap=[[dw0.ap[0][0] * 32, 4], [1, 400]])
                    nc.sync.dma_start(out=d_dscr[b], in_=gsrc)
                    bb = dpp.tile([128, 400], F32, tag="bb")
                    for j in range(4):
                        bsrc = bass.AP(tensor=d_dscr[b].tensor,
                                       offset=d_dscr[b].offset + j * 400,
                                       ap=[[0, 32], [1, 400]])
                        nc.sync.dma_start(out=bb[32 * j:32 * j + 32, :], in_=bsrc)
                    nc.vector.tensor_tensor(out=sq[:, 400 * bh:400 * bh + 400],
                                            in0=hc[:], in1=bb[:], op=OP.mult)
                if PHASE < 5:
                    continue
                # Wo + residual
                po = psB.tile([128, 1024], F32, tag="qkv")
                sqv = sq[:].rearrange("p (b g y) -> p b g y", b=4, g=2)
                for part in range(2):
                    oc = 512 * part
                    for g in range(2):
                        nc.tensor.matmul(po[:, oc:oc + 400], w_op[l][g],
                                         sqv[:, 2 * part:2 * part + 2, g, :],
                                         start=(g == 0), stop=(g == 1))
                po0 = po[:]
                posrc = bass.AP(tensor=po0.tensor, offset=po0.offset,
                                ap=[list(po0.ap[0]), [512, 2], [1, 400]])
                nc.vector.tensor_tensor(out=HP1[:, t0:t0 + 800], in0=posrc,
                                        in1=XTn[:, t0:t0 + 800],
                                        op=OP.add)
                if PHASE >= 6:
                    quad_stats(q)

            if PHASE >= 6:
                bn_sync(l, 1, HP1[:], 2 * l)

            # ===== FFN =====
            for q in range(NQ if PHASE >= 7 else 0):
                t0 = q * TQ
                rls = []
                for c in range(4):
                    pf = psB.tile([128, 1024], F32, tag="qkv")
                    nc.tensor.matmul(pf[:, 0:512], w_1[l][:, 128 * c:128 * c + 128],
                                     XS[:, t0:t0 + 512], start=True, stop=True)
                    nc.tensor.matmul(pf[:, 512:800],
                                     w_1[l][:, 128 * c:128 * c + 128],
                                     XS[:, t0 + 512:t0 + 800],
                                     start=True, stop=True)
                    rl = rlp.tile([128, 800], BF16, tag=f"rl{c}")
                    nc.vector.tensor_scalar(out=rl[:], in0=pf[:, 0:800],
                                            scalar1=c_b1[l][:, c:c + 1],
                                            scalar2=0.0, op0=OP.add, op1=OP.max)
                    rls.append(rl)
                pw = psB.tile([128, 1024], F32, tag="qkv")
                for (oc, off, sz) in ((0, 0, 512), (512, 512, 288)):
                    for c in range(4):
                        nc.tensor.matmul(pw[:, oc:oc + sz], w_2[l][c],
                                         rls[c][:, off:off + sz],
                                         start=(c == 0), stop=(c == 3))
                nc.vector.tensor_tensor(out=HP1[:, t0:t0 + 800],
                                        in0=pw[:, 0:800],
                                        in1=XTn[:, t0:t0 + 800],
                                        op=OP.add)
                quad_stats(q)

            if PHASE >= 7:
                bn_sync(l, 2, HP1[:], 2 * l + 1)

        # ---------- outputs (per quad to overlap with last BN) ----------
        for qq in range(NQ):
            o0 = qq * TQ
            nc.sync.dma_start(out=d_oh[:, o0:o0 + TQ], in_=XTn[:, o0:o0 + TQ])
            xv = XTn[:, o0:o0 + TQ].rearrange("p (b n) -> p b n", b=4)
            nc.vector.tensor_reduce(out=s_mean[:, 4 * qq:4 * qq + 4], in_=xv,
                                    axis=mybir.AxisListType.X, op=OP.add)
        nc.vector.tensor_scalar(out=s_mean[:], in0=s_mean[:],
                                scalar1=1.0 / N, scalar2=None, op0=OP.mult)
        nc.sync.dma_start(out=d_om[:], in_=s_mean[:])
        _es.close()

    nc.finalize()
    return nc


def _prep_weights(inputs):
    """Host-side weight preprocessing -> per-core input dict (replicated)."""
    Wemb = np.asarray(inputs["Wemb"], np.float32)
    bemb = np.asarray(inputs["bemb"], np.float32).reshape(D, 1)
    Wq = np.asarray(inputs["Wq"], np.float32) * 0.25  # fold 1/sqrt(KD)
    Wk = np.asarray(inputs["Wk"], np.float32)
    Wv = np.asarray(inputs["Wv"], np.float32)
    Wo = np.asarray(inputs["Wo"], np.float32)
    W1 = np.asarray(inputs["W1"], np.float32)
    W2 = np.asarray(inputs["W2"], np.float32)
    b1 = np.asarray(inputs["b1"], np.float32)
    bn1_g = np.asarray(inputs["bn1_g"], np.float32)
    bn1_b = np.asarray(inputs["bn1_b"], np.float32)
    bn2_g = np.asarray(inputs["bn2_g"], np.float32)
    bn2_b = np.asarray(inputs["bn2_b"], np.float32)

    wqp = np.zeros((L, 2, D, 128), np.float32)
    wkp = np.zeros((L, 2, D, 128), np.float32)
    wop = np.zeros((L, 2, 128, D), np.float32)
    wvd = np.zeros((L, D, 128), np.float32)
    for l in range(L):
        for h in range(H):
            g, j = divmod(h, 4)
            wqp[l, g, :, 32 * j:32 * j + KD] = Wq[l, h]
            wkp[l, g, :, 32 * j:32 * j + KD] = Wk[l, h]
            wop[l, g, 32 * j:32 * j + KD, :] = Wo[l, h]
            wvd[l, :, KD * h:KD * h + KD] = Wv[l, h]
    w2c = np.zeros((L, 4, 128, D), np.float32)
    for c in range(4):
        w2c[:, c] = W2[:, 128 * c:128 * c + 128, :]
    b1t = np.zeros((L, 128, 4), np.float32)
    for c in range(4):
        b1t[:, :, c] = b1[:, 128 * c:128 * c + 128]
    bnp = np.stack([bn1_g, bn1_b, bn2_g, bn2_b], axis=2)  # [L, D, 4]

    f16 = np.float16
    return {
        "wemb": Wemb, "bemb": bemb, "wqp": wqp.astype(f16),
        "wkp": wkp.astype(f16), "wvd": wvd.astype(f16),
        "wop": wop.astype(f16), "w1": W1.astype(f16), "w2": w2c.astype(f16),
        "b1t": b1t, "bnp": bnp,
    }


def _install_ntff_hook():
    """Make trace=True work under axon: inject antenv.axon_hooks and register
    the ctypes NTFF profiling hook against libaxon_pjrt.so."""
    import types
    import importlib
    try:
        import antenv
        if hasattr(antenv, "axon_hooks"):
            return True
        mod = types.ModuleType("antenv.axon_hooks")
        state = {"hook": None}
        mod.set_axon_ntff_profile_hook = lambda h: state.__setitem__("hook", h)
        mod.get_axon_ntff_profile_hook = lambda: state["hook"]
        sys.modules["antenv.axon_hooks"] = mod
        antenv.axon_hooks = mod
        sys.path.insert(0, "/root/.axon_site")
        tb = importlib.import_module("trn_agent_boot.trn_boot")
        hook = tb._ntff_profile_via_ctypes("/opt/axon/libaxon_pjrt.so")
        if hook is None:
            return False
        mod.set_axon_ntff_profile_hook(hook)
        return True
    except Exception:
        return False


def kernel(**inputs):
    sys.path.insert(0, "/opt/trn_rl_repo")
    from concourse.bass_utils import run_bass_kernel_spmd

    if "nc" not in _BUILD_CACHE:
        _BUILD_CACHE["nc"] = _build_bass()
    nc = _BUILD_CACHE["nc"]

    wd = _prep_weights(inputs)
    x = np.asarray(inputs["x"], np.float32)
    in_maps = []
    for core in range(NCORES):
        m = dict(wd)
        xs = x[core * BL:(core + 1) * BL]          # [BL, N, 2]
        m["x"] = np.ascontiguousarray(
            xs.transpose(2, 0, 1).reshape(NODE_DIM, T))
        in_maps.append(m)

    trace = bool(int(os.environ.get("KERNEL_TRACE", "0")))
    if trace:
        trace = _install_ntff_hook()
    res = run_bass_kernel_spmd(nc, in_maps, core_ids=list(range(NCORES)),
                               trace=trace)
    _BUILD_CACHE["last_result"] = res

    h = np.empty((B, N, D), np.float32)
    hm = np.empty((B, D), np.float32)
    for core in range(NCORES):
        oh = res.results[core]["oh"]           # [D, T]
        om = res.results[core]["om"]           # [D, BL]
        h[core * BL:(core + 1) * BL] = (
            oh.reshape(D, BL, N).transpose(1, 2, 0))
        hm[core * BL:(core + 1) * BL] = om.T
    return (h, hm)


if __name__ == "__main__":
    nc = _build_bass()
    print("build OK, instructions:", len(nc.inst_map))


# revision 28
# speedup vs baseline: 1.2280x; 1.2280x over previous
"""Trainium2 Bass kernel for nn_AttentionModel2 (Kool-style attention encoder).

Model (per reference):
  h = x @ Wemb + bemb                      # [B=256, N=200, D=128]
  3 layers of:
    h = BN1(h + MHA(h))                    # BatchNorm1d training mode (global stats)
    h = BN2(h + FFN(BN1-output))           # FFN = relu(h W1 + b1) W2 + b2
  return (h, h.mean(axis=1))

Sharding: data-parallel over batch, 32 instances per core x 8 cores.
BatchNorm statistics are global over all 256*200 tokens -> tiny AllGather of
per-core (sum, sumsq) partials, 6x (2 BN per layer).

Layout: activations live transposed [D=128 partitions, 6400 tokens] on-chip.
b2 is dropped entirely: a per-channel constant shift cancels exactly in BN2.
"""

import os
import sys
import numpy as np

B, N, NODE_DIM = 256, 200, 2
H, D, L, FF = 8, 128, 3, 512
KD = D // H  # 16
EPS = 1e-5
NCORES = 8
BL = B // NCORES          # 32 instances per core
T = BL * N                # 6400 tokens per core
NQ = 8                    # quads (4 instances each)
TQ = 4 * N                # 800 tokens per quad

_BUILD_CACHE = {}


def _build_bass():
    PHASE = int(os.environ.get("KERNEL_PHASE", "8"))
    sys.path.insert(0, "/opt/trn_rl_repo")
    import concourse.bass as bass
    import concourse.mybir as mybir
    import concourse.tile as tile
    from concourse import bacc

    F32 = mybir.dt.float32
    BF16 = mybir.dt.float16  # fp16 for weights/activations (better mantissa)
    EX16 = mybir.dt.bfloat16  # bf16 where range matters (expS can reach e^85)
    F32R = mybir.dt.float32r
    AF = mybir.ActivationFunctionType
    OP = mybir.AluOpType

    nc = bacc.Bacc("TRN2", target_bir_lowering=False)

    # ---------------- DRAM I/O ----------------
    d_x = nc.dram_tensor("x", [NODE_DIM, T], F32, kind="ExternalInput")
    d_wemb = nc.dram_tensor("wemb", [NODE_DIM, D], F32, kind="ExternalInput")
    d_bemb = nc.dram_tensor("bemb", [D, 1], F32, kind="ExternalInput")
    F16_ = mybir.dt.float16
    d_wqp = nc.dram_tensor("wqp", [L, 2, D, 128], F16_, kind="ExternalInput")
    d_wkp = nc.dram_tensor("wkp", [L, 2, D, 128], F16_, kind="ExternalInput")
    d_wvd = nc.dram_tensor("wvd", [L, D, 128], F16_, kind="ExternalInput")
    d_wop = nc.dram_tensor("wop", [L, 2, 128, D], F16_, kind="ExternalInput")
    d_w1 = nc.dram_tensor("w1", [L, D, FF], F16_, kind="ExternalInput")
    d_w2 = nc.dram_tensor("w2", [L, 4, 128, D], F16_, kind="ExternalInput")
    d_b1t = nc.dram_tensor("b1t", [L, 128, 4], F32, kind="ExternalInput")
    d_bnp = nc.dram_tensor("bnp", [L, D, 4], F32, kind="ExternalInput")

    d_oh = nc.dram_tensor("oh", [D, T], F32, kind="ExternalOutput")
    d_om = nc.dram_tensor("om", [D, BL], F32, kind="ExternalOutput")

    # collective buffers (one pair per BN instance)
    cc_in = [nc.dram_tensor(f"cc_in{k}", [1, 2 * D], F32) for k in range(2 * L)]
    cc_out = [
        nc.dram_tensor(f"cc_out{k}", [NCORES, 2 * D], F32, addr_space="Shared")
        for k in range(2 * L)
    ]
    # scratch for softmax denominators (per instance: 4 rows x 400)
    d_dscr = nc.dram_tensor("dscr", [BL, 4, 400], F32)

    # ---------------- persistent SBUF ----------------
    def sb(name, shape, dt):
        return nc.alloc_sbuf_tensor(name, shape, dt).ap()

    XTn = sb("XTn", [D, T], F32)           # spine (normalized h.T)
    XS = sb("XS", [D, T], BF16)            # bf16 shadow for matmul rhs
    HP1 = sb("HP1", [D, T], F32)           # pre-BN spine
    VPall = sb("VPall", [128, 2 * BL * 136], EX16)  # V' per (b,c): 8 heads x 17
    xS2 = sb("xS2", [NODE_DIM, T], F32)

    w_emb = sb("w_emb", [NODE_DIM, D], F32)
    w_qp = [[sb(f"w_qp{l}_{g}", [D, 128], BF16) for g in range(2)] for l in range(L)]
    w_kp = [[sb(f"w_kp{l}_{g}", [D, 128], BF16) for g in range(2)] for l in range(L)]
    w_vd = [sb(f"w_vd{l}", [D, 128], BF16) for l in range(L)]
    w_op = [[sb(f"w_op{l}_{g}", [128, D], BF16) for g in range(2)] for l in range(L)]
    w_1 = [sb(f"w_1_{l}", [D, FF], BF16) for l in range(L)]
    w_2 = [[sb(f"w_2_{l}_{c}", [128, D], BF16) for c in range(4)] for l in range(L)]

    c_bemb = sb("c_bemb", [D, 1], F32)
    c_b1 = [sb(f"c_b1_{l}", [128, 4], F32) for l in range(L)]
    c_bnp = [sb(f"c_bnp_{l}", [D, 4], F32) for l in range(L)]

    # small stat tensors
    s_stats = sb("s_stats", [D, 16, 6], F32)
    s_mv = sb("s_mv", [D, 2], F32)
    s_sums = sb("s_sums", [D, 2], F32)
    s_tmp = sb("s_tmp", [D, 2], F32)
    s_gat = sb("s_gat", [D, NCORES, 2], F32)
    s_gl = sb("s_gl", [D, 2], F32)         # global (sum, sumsq) -> (mean, var)
    s_sc = sb("s_sc", [D, 1], F32)         # BN scale
    s_sh = sb("s_sh", [D, 1], F32)         # BN shift
    s_mean = sb("s_mean", [D, BL], F32)
    s_eps = sb("s_eps", [D, 1], F32)

    CHUNKS = [(i * 512, 512) for i in range(12)] + [(6144, 256)]

    from contextlib import ExitStack
    _es = ExitStack()
    with tile.TileContext(nc) as tc:
        psA = _es.enter_context(tc.tile_pool(name="psA", bufs=1, space="PSUM"))
        psB = _es.enter_context(tc.tile_pool(name="psB", bufs=2, space="PSUM"))
        psH = _es.enter_context(tc.tile_pool(name="psH", bufs=2, space="PSUM"))
        sbp = _es.enter_context(tc.tile_pool(name="sbp", bufs=2))
        dpp = _es.enter_context(tc.tile_pool(name="dpp", bufs=4))
        exp_pool = _es.enter_context(tc.tile_pool(name="exp_pool", bufs=3))
        rlp = _es.enter_context(tc.tile_pool(name="rlp", bufs=1))

        # ---------- load weights ----------
        def load_round(dram_ap, sbuf_ap, cols):
            nc.sync.dma_start(out=sbuf_ap, in_=dram_ap)

        load_round(d_wemb[:], w_emb, D)
        for l in range(L):
            for g in range(2):
                load_round(d_wqp[l, g], w_qp[l][g], 128)
                load_round(d_wkp[l, g], w_kp[l][g], 128)
                load_round(d_wop[l, g], w_op[l][g], D)
            load_round(d_wvd[l], w_vd[l], 128)
            load_round(d_w1[l], w_1[l], FF)
            for c in range(4):
                load_round(d_w2[l, c], w_2[l][c], D)
            nc.sync.dma_start(out=c_b1[l], in_=d_b1t[l])
            nc.sync.dma_start(out=c_bnp[l], in_=d_bnp[l])
        nc.sync.dma_start(out=c_bemb[:], in_=d_bemb[:])

        nc.vector.memset(s_eps[:], EPS)
        # zero-init all PSUM pool slots (dead rows of M=17 col-tiled matmuls
        # are never written; stale Inf would poison Wo via 0*Inf)
        for _pool, _shape, _n in ((psA, [128, 1024], 1), (psB, [128, 1024], 2),
                                  (psH, [128, 400], 2)):
            for _i in range(_n):
                _pt = _pool.tile(_shape, F32, tag="sc" if _pool is psA else
                                 ("qkv" if _pool is psB else "h"))
                nc.vector.memset(_pt[:], 0.0)
        # V' ones columns (one-time; data evacs never touch col 16 of each 17)
        ones_ap = bass.AP(
            tensor=VPall.tensor,
            offset=VPall.offset + 16,
            ap=[[VPall.ap[0][0], 128], [136, 2 * BL], [17, 8]],
        )
        nc.vector.memset(ones_ap, 1.0)

        # ---------- embed ----------
        nc.sync.dma_start(out=xS2[:], in_=d_x[:])
        for (off, sz) in CHUNKS:
            pe = psB.tile([128, 1024], F32, tag="qkv")
            nc.tensor.matmul(pe[:, 0:sz], w_emb, xS2[:, off:off + sz],
                             start=True, stop=True)
            nc.vector.tensor_scalar(out=XTn[:, off:off + sz], in0=pe[:, 0:sz],
                                    scalar1=c_bemb[:, 0:1], scalar2=None,
                                    op0=OP.add)
            nc.vector.tensor_scalar(out=XS[:, off:off + sz], in0=pe[:, 0:sz],
                                    scalar1=c_bemb[:, 0:1], scalar2=None,
                                    op0=OP.add)

        # ---------- BN helper ----------
        def quad_stats(q):
            # incremental stats on HP1 quad (800 = 512 + 288)
            t0 = q * TQ
            nc.vector.bn_stats(s_stats[:, 2 * q, :], HP1[:, t0:t0 + 512])
            nc.vector.bn_stats(s_stats[:, 2 * q + 1, :], HP1[:, t0 + 512:t0 + 800])

        def bn_sync(l, which, src_ap, k):
            nc.vector.bn_aggr(s_mv[:], s_stats[:])
            # partial (sum, sumsq)
            nc.vector.tensor_scalar(out=s_sums[:, 0:1], in0=s_mv[:, 0:1],
                                    scalar1=float(T), scalar2=None, op0=OP.mult)
            nc.vector.tensor_tensor(out=s_tmp[:, 0:1], in0=s_mv[:, 0:1],
                                    in1=s_mv[:, 0:1], op=OP.mult)
            nc.vector.tensor_tensor(out=s_tmp[:, 1:2], in0=s_mv[:, 1:2],
                                    in1=s_tmp[:, 0:1], op=OP.add)
            nc.vector.tensor_scalar(out=s_sums[:, 1:2], in0=s_tmp[:, 1:2],
                                    scalar1=float(T), scalar2=None, op0=OP.mult)
            # DMA out [D,2] -> [1, 2D] (d-major)
            src = bass.AP(tensor=s_sums.tensor, offset=s_sums.offset,
                          ap=[[2, D], [1, 2]])
            nc.sync.dma_start(out=cc_in[k][:], in_=src)
            nc.gpsimd.collective_compute(
                "AllGather", OP.bypass,
                ins=[cc_in[k][:]], outs=[cc_out[k][:]],
                replica_groups=[list(range(NCORES))],
            )
            gsrc = bass.AP(tensor=cc_out[k][:].tensor, offset=cc_out[k][:].offset,
                           ap=[[2, D], [2 * D, NCORES], [1, 2]])
            nc.sync.dma_start(out=s_gat[:], in_=gsrc)
            nc.vector.tensor_reduce(out=s_gl[:],
                                    in_=s_gat[:].rearrange("p c s -> p s c"),
                                    axis=mybir.AxisListType.X, op=OP.add)
            inv = 1.0 / (B * N)
            nc.vector.tensor_scalar(out=s_gl[:, 0:1], in0=s_gl[:, 0:1],
                                    scalar1=inv, scalar2=None, op0=OP.mult)
            nc.vector.tensor_scalar(out=s_gl[:, 1:2], in0=s_gl[:, 1:2],
                                    scalar1=inv, scalar2=None, op0=OP.mult)
            # var = E[x^2] - mean^2
            nc.vector.tensor_tensor(out=s_tmp[:, 0:1], in0=s_gl[:, 0:1],
                                    in1=s_gl[:, 0:1], op=OP.mult)
            nc.vector.tensor_tensor(out=s_tmp[:, 1:2], in0=s_gl[:, 1:2],
                                    in1=s_tmp[:, 0:1], op=OP.subtract)
            # rstd = exp(-0.5*ln(var+eps)) (stays in natural_log_exp table set)
            nc.scalar.activation(out=s_tmp[:, 1:2], in_=s_tmp[:, 1:2],
                                 func=AF.Ln, bias=s_eps[:, 0:1], scale=1.0)
            nc.scalar.activation(out=s_tmp[:, 1:2], in_=s_tmp[:, 1:2],
                                 func=AF.Exp, bias=0.0, scale=-0.5)
            gcol = 0 if which == 1 else 2
            nc.vector.tensor_tensor(out=s_sc[:, 0:1], in0=s_tmp[:, 1:2],
                                    in1=c_bnp[l][:, gcol:gcol + 1], op=OP.mult)
            nc.vector.tensor_tensor(out=s_sh[:, 0:1], in0=s_gl[:, 0:1],
                                    in1=s_sc[:, 0:1], op=OP.mult)
            nc.vector.tensor_tensor(out=s_sh[:, 0:1],
                                    in0=c_bnp[l][:, gcol + 1:gcol + 2],
                                    in1=s_sh[:, 0:1], op=OP.subtract)
            # normalize per quad: XS (f16 shadow) first to unblock matmuls,
            # then XTn (fp32 spine)
            for qq in range(NQ):
                o0 = qq * TQ
                nc.vector.tensor_scalar(out=XS[:, o0:o0 + TQ],
                                        in0=src_ap[:, o0:o0 + TQ],
                                        scalar1=s_sc[:, 0:1], scalar2=s_sh[:, 0:1],
                                        op0=OP.mult, op1=OP.add)
                nc.vector.tensor_scalar(out=XTn[:, o0:o0 + TQ],
                                        in0=src_ap[:, o0:o0 + TQ],
                                        scalar1=s_sc[:, 0:1], scalar2=s_sh[:, 0:1],
                                        op0=OP.mult, op1=OP.add)

        # ---------- layers ----------
        for l in range(L if PHASE >= 8 else 1):
            # ===== attention =====
            for q in range(NQ if PHASE >= 1 else 0):
                t0 = q * TQ
                # Q, K projection (padded 4-head groups)
                QK = {}
                for (nm, wg) in (("q", w_qp[l]), ("k", w_kp[l])):
                    for g in range(2):
                        pp = psB.tile([128, 1024], F32, tag="qkv")
                        nc.tensor.matmul(pp[:, 0:512], wg[g], XS[:, t0:t0 + 512],
                                         start=True, stop=True)
                        nc.tensor.matmul(pp[:, 512:800], wg[g],
                                         XS[:, t0 + 512:t0 + 800],
                                         start=True, stop=True)
                        sbt = sbp.tile([128, 800], BF16, tag=f"{nm}{g}")
                        nc.vector.tensor_copy(sbt[:], pp[:, 0:800])
                        QK[(nm, g)] = sbt
                # V' production
                for bh in range(4):
                    b = 4 * q + bh
                    pv = psB.tile([128, 256], F32, tag="qkv")
                    for c in range(2):
                        kl = 128 if c == 0 else 72
                        nc.tensor.matmul(
                            pv[0:kl, 128 * c:128 * c + 128],
                            XS[:, t0 + 200 * bh + 128 * c:
                               t0 + 200 * bh + 128 * c + kl],
                            w_vd[l], start=True, stop=True)
                    for c in range(2):
                        kl = 128 if c == 0 else 72
                        slot = 2 * b + c
                        dst = bass.AP(
                            tensor=VPall.tensor,
                            offset=VPall.offset + slot * 136,
                            ap=[[VPall.ap[0][0], kl], [17, 8], [1, 16]])
                        src = pv[0:kl, 128 * c:128 * c + 128].rearrange(
                            "p (h k) -> p h k", h=8)
                        nc.vector.tensor_copy(dst, src)

                sq = sbp.tile([128, 1600], BF16, tag="sq")
                for bh in range(4 if PHASE >= 2 else 0):
                    b = 4 * q + bh
                    Hb = psH.tile([128, 400], F32, tag="h")
                    for g in range(2):
                        exs = []
                        for jp in range(2):
                            sc = psA.tile([128, 1024], F32, tag="sc")
                            for jj in range(2):
                                j = 2 * jp + jj
                                for c in range(2):
                                    kl = 128 if c == 0 else 72
                                    nc.tensor.matmul(
                                        sc[0:kl, 512 * jj + 200 * c:
                                           512 * jj + 200 * c + 200],
                                        QK[("k", g)][32 * j:32 * j + 16,
                                                     200 * bh + 128 * c:
                                                     200 * bh + 128 * c + kl],
                                        QK[("q", g)][32 * j:32 * j + 16,
                                                     200 * bh:200 * bh + 200],
                                        start=True, stop=True,
                                        tile_position=(32 * j, 0))
                            ex = exp_pool.tile([128, 2, 2, 200], EX16, tag="ex")
                            sc0 = sc[:]
                            esrc = bass.AP(
                                tensor=sc0.tensor, offset=sc0.offset,
                                ap=[list(sc0.ap[0]), [512, 2], [200, 2], [1, 200]])
                            nc.scalar.activation(out=ex[:], in_=esrc, func=AF.Exp)
                            exs.append(ex)
                        # attnV: c-major rounds so drains overlap across heads
                        for c in range(2 if PHASE >= 3 else 0):
                            kl = 128 if c == 0 else 72
                            slot = 2 * b + c
                            for jp in range(2):
                                for jj in range(2):
                                    j = 2 * jp + jj
                                    vap = bass.AP(
                                        tensor=VPall.tensor,
                                        offset=VPall.offset + slot * 136
                                        + 17 * (4 * g + j),
                                        ap=[[VPall.ap[0][0], kl], [1, 17]])
                                    nc.tensor.matmul(
                                        Hb[32 * j:32 * j + 17,
                                           200 * g:200 * g + 200],
                                        vap, exs[jp][0:kl, jj, c, :],
                                        start=(c == 0), stop=(c == 1),
                                        tile_position=(0, 32 * j))
                    if PHASE < 4:
                        continue
                    # free Hb fast: copy to SBUF, chain runs off the copy
                    hc = dpp.tile([128, 400], F32, tag="hc")
                    nc.vector.tensor_copy(hc[:], Hb[:])
                    dw = dpp.tile([128, 400], F32, tag="draw")
                    nc.vector.reciprocal_approx_fast(dw[:], hc[:])
                    dw0 = dw[:]
                    gsrc = bass.AP(tensor=dw0.tensor,
                                   offset=dw0.offset + 16 * dw0.ap[0][0],
                                   ap=[[dw0.ap[0][0] * 32, 4], [1, 400]])
                    nc.sync.dma_start(out=d_dscr[b], in_=gsrc)
                    bb = dpp.tile([128, 400], F32, tag="bb")
                    for j in range(4):
                        bsrc = bass.AP(tensor=d_dscr[b].tensor,
                                       offset=d_dscr[b].offset + j * 400,
                                       ap=[[0, 32], [1, 400]])
                        nc.sync.dma_start(out=bb[32 * j:32 * j + 32, :], in_=bsrc)
                    nc.vector.tensor_tensor(out=sq[:, 400 * bh:400 * bh + 400],
                                            in0=hc[:], in1=bb[:], op=OP.mult)
                if PHASE < 5:
                    continue
                # Wo + residual
                po = psB.tile([128, 1024], F32, tag="qkv")
                sqv = sq[:].rearrange("p (b g y) -> p b g y", b=4, g=2)
                for part in range(2):
                    oc = 512 * part
                    for g in range(2):
                        nc.tensor.matmul(po[:, oc:oc + 400], w_op[l][g],
                                         sqv[:, 2 * part:2 * part + 2, g, :],
                                         start=(g == 0), stop=(g == 1))
                po0 = po[:]
                posrc = bass.AP(tensor=po0.tensor, offset=po0.offset,
                                ap=[list(po0.ap[0]), [512, 2], [1, 400]])
                nc.vector.tensor_tensor(out=HP1[:, t0:t0 + 800], in0=posrc,
                                        in1=XTn[:, t0:t0 + 800],
                                        op=OP.add)
                if PHASE >= 6:
                    quad_stats(q)

            if PHASE >= 6:
                bn_sync(l, 1, HP1[:], 2 * l)

            # ===== FFN =====
            for q in range(NQ if PHASE >= 7 else 0):
                t0 = q * TQ
                rls = []
                for c in range(4):
                    pf = psB.tile([128, 1024], F32, tag="qkv")
                    nc.tensor.matmul(pf[:, 0:512], w_1[l][:, 128 * c:128 * c + 128],
                                     XS[:, t0:t0 + 512], start=True, stop=True)
                    nc.tensor.matmul(pf[:, 512:800],
                                     w_1[l][:, 128 * c:128 * c + 128],
                                     XS[:, t0 + 512:t0 + 800],
                                     start=True, stop=True)
                    rl = rlp.tile([128, 800], BF16, tag=f"rl{c}")
                    if c % 2 == 0:
                        nc.scalar.activation(out=rl[:], in_=pf[:, 0:800],
                                             func=AF.Relu,
                                             bias=c_b1[l][:, c:c + 1], scale=1.0)
                    else:
                        nc.vector.tensor_scalar(out=rl[:], in0=pf[:, 0:800],
                                                scalar1=c_b1[l][:, c:c + 1],
                                                scalar2=0.0, op0=OP.add, op1=OP.max)
                    rls.append(rl)
                pw = psB.tile([128, 1024], F32, tag="qkv")
                for (oc, off, sz) in ((0, 0, 512), (512, 512, 288)):
                    for c in range(4):
                        nc.tensor.matmul(pw[:, oc:oc + sz], w_2[l][c],
                                         rls[c][:, off:off + sz],
                                         start=(c == 0), stop=(c == 3))
                nc.vector.tensor_tensor(out=HP1[:, t0:t0 + 800],
                                        in0=pw[:, 0:800],
                                        in1=XTn[:, t0:t0 + 800],
                                        op=OP.add)
                quad_stats(q)

            if PHASE >= 7:
                bn_sync(l, 2, HP1[:], 2 * l + 1)

        # ---------- outputs (per quad to overlap with last BN) ----------
        for qq in range(NQ):
            o0 = qq * TQ
            nc.sync.dma_start(out=d_oh[:, o0:o0 + TQ], in_=XTn[:, o0:o0 + TQ])
            xv = XTn[:, o0:o0 + TQ].rearrange("p (b n) -> p b n", b=4)
            nc.vector.tensor_reduce(out=s_mean[:, 4 * qq:4 * qq + 4], in_=xv,
                                    axis=mybir.AxisListType.X, op=OP.add)
        nc.vector.tensor_scalar(out=s_mean[:], in0=s_mean[:],
                                scalar1=1.0 / N, scalar2=None, op0=OP.mult)
        nc.sync.dma_start(out=d_om[:], in_=s_mean[:])
        _es.close()

    nc.finalize()
    return nc


def _prep_weights(inputs):
    """Host-side weight preprocessing -> per-core input dict (replicated)."""
    Wemb = np.asarray(inputs["Wemb"], np.float32)
    bemb = np.asarray(inputs["bemb"], np.float32).reshape(D, 1)
    Wq = np.asarray(inputs["Wq"], np.float32) * 0.25  # fold 1/sqrt(KD)
    Wk = np.asarray(inputs["Wk"], np.float32)
    Wv = np.asarray(inputs["Wv"], np.float32)
    Wo = np.asarray(inputs["Wo"], np.float32)
    W1 = np.asarray(inputs["W1"], np.float32)
    W2 = np.asarray(inputs["W2"], np.float32)
    b1 = np.asarray(inputs["b1"], np.float32)
    bn1_g = np.asarray(inputs["bn1_g"], np.float32)
    bn1_b = np.asarray(inputs["bn1_b"], np.float32)
    bn2_g = np.asarray(inputs["bn2_g"], np.float32)
    bn2_b = np.asarray(inputs["bn2_b"], np.float32)

    wqp = np.zeros((L, 2, D, 128), np.float32)
    wkp = np.zeros((L, 2, D, 128), np.float32)
    wop = np.zeros((L, 2, 128, D), np.float32)
    wvd = np.zeros((L, D, 128), np.float32)
    for l in range(L):
        for h in range(H):
            g, j = divmod(h, 4)
            wqp[l, g, :, 32 * j:32 * j + KD] = Wq[l, h]
            wkp[l, g, :, 32 * j:32 * j + KD] = Wk[l, h]
            wop[l, g, 32 * j:32 * j + KD, :] = Wo[l, h]
            wvd[l, :, KD * h:KD * h + KD] = Wv[l, h]
    w2c = np.zeros((L, 4, 128, D), np.float32)
    for c in range(4):
        w2c[:, c] = W2[:, 128 * c:128 * c + 128, :]
    b1t = np.zeros((L, 128, 4), np.float32)
    for c in range(4):
        b1t[:, :, c] = b1[:, 128 * c:128 * c + 128]
    bnp = np.stack([bn1_g, bn1_b, bn2_g, bn2_b], axis=2)  # [L, D, 4]

    f16 = np.float16
    return {
        "wemb": Wemb, "bemb": bemb, "wqp": wqp.astype(f16),
        "wkp": wkp.astype(f16), "wvd": wvd.astype(f16),
        "wop": wop.astype(f16), "w1": W1.astype(f16), "w2": w2c.astype(f16),
        "b1t": b1t, "bnp": bnp,
    }


def _install_ntff_hook():
    """Make trace=True work under axon: inject antenv.axon_hooks and register
    the ctypes NTFF profiling hook against libaxon_pjrt.so."""
    import types
    import importlib
    try:
        import antenv
        if hasattr(antenv, "axon_hooks"):
            return True
        mod = types.ModuleType("antenv.axon_hooks")
        state = {"hook": None}
        mod.set_axon_ntff_profile_hook = lambda h: state.__setitem__("hook", h)
        mod.get_axon_ntff_profile_hook = lambda: state["hook"]
        sys.modules["antenv.axon_hooks"] = mod
        antenv.axon_hooks = mod
        sys.path.insert(0, "/root/.axon_site")
        tb = importlib.import_module("trn_agent_boot.trn_boot")
        hook = tb._ntff_profile_via_ctypes("/opt/axon/libaxon_pjrt.so")
        if hook is None:
            return False
        mod.set_axon_ntff_profile_hook(hook)
        return True
    except Exception:
        return False


def kernel(**inputs):
    sys.path.insert(0, "/opt/trn_rl_repo")
    from concourse.bass_utils import run_bass_kernel_spmd

    if "nc" not in _BUILD_CACHE:
        _BUILD_CACHE["nc"] = _build_bass()
    nc = _BUILD_CACHE["nc"]

    wd = _prep_weights(inputs)
    x = np.asarray(inputs["x"], np.float32)
    in_maps = []
    for core in range(NCORES):
        m = dict(wd)
        xs = x[core * BL:(core + 1) * BL]          # [BL, N, 2]
        m["x"] = np.ascontiguousarray(
            xs.transpose(2, 0, 1).reshape(NODE_DIM, T))
        in_maps.append(m)

    trace = bool(int(os.environ.get("KERNEL_TRACE", "0")))
    if trace:
        trace = _install_ntff_hook()
    res = run_bass_kernel_spmd(nc, in_maps, core_ids=list(range(NCORES)),
                               trace=trace)
    _BUILD_CACHE["last_result"] = res

    h = np.empty((B, N, D), np.float32)
    hm = np.empty((B, D), np.float32)
    for core in range(NCORES):
        oh = res.results[core]["oh"]           # [D, T]
        om = res.results[core]["om"]           # [D, BL]
        h[core * BL:(core + 1) * BL] = (
            oh.reshape(D, BL, N).transpose(1, 2, 0))
        hm[core * BL:(core + 1) * BL] = om.T
    return (h, hm)


if __name__ == "__main__":
    nc = _build_bass()
    print("build OK, instructions:", len(nc.inst_map))
